# revision 12
# baseline (speedup 1.0000x reference)
"""Trainium2 Bass kernel for CenterLoss (loss + segment-mean center update).

Strategy (per sharding hint): centers table [1M, 128] is sharded row-wise
across 8 NeuronCores (125000 rows each). Host routes (feature, label) pairs
to the owning shard (sorted by label), each core then:

  Phase A: bulk-copies its centers shard to the output shard (DRAM->DRAM DMA,
           the unavoidable memory-roofline term: 64MB read + 64MB write/core).
  Phase B: computes an exclusive prefix sum of the routed (label-sorted)
           features via triangular matmuls on the PE (carry chained through a
           broadcast matmul), stores it to a DRAM scratch, then per 128-item
           chunk: segment sum = prefix[hi] - prefix[lo] (two indirect row
           gathers), mean blend new_c = 0.5*c + 0.5*mean, and an indirect
           row-scatter of the blended rows over the copied shard. Loss
           partials (sum of squared diffs vs gathered old centers) are
           accumulated per partition and reduced on host.

Padding items carry gather/scatter index = SHARD (out of bounds -> skipped
via bounds_check) and zeroed features, so they contribute nothing.
"""

import numpy as np
from contextlib import ExitStack

# Problem geometry (hardcoded per contract)
NUM_CLASSES = 1_000_000
FEAT_DIM = 128
BATCH = 16384
LAMBDA_C = 1.0
ALPHA = 0.5  # update blend: new = (1-ALPHA)*c + ALPHA*mean
N_CORES = 8
SHARD = NUM_CLASSES // N_CORES
P = 128

_PROGRAM_CACHE: dict = {}


# ---------------------------------------------------------------------------
# Host-side routing
# ---------------------------------------------------------------------------

def _route(features, centers, labels, shard=SHARD, n_cores=N_CORES):
    """Route (feature, label) pairs to owning shards; build per-core inputs."""
    labels = np.asarray(labels).astype(np.int64)
    features = np.ascontiguousarray(np.asarray(features, dtype=np.float32))
    centers = np.asarray(centers, dtype=np.float32)

    owner = labels // shard
    counts = np.bincount(owner, minlength=n_cores)
    cap = max(P, int(-(-int(counts.max()) // P)) * P)
    nch = cap // P

    tri = np.triu(np.ones((P, P), np.float32))  # lhsT: T[j,i]=1 for j<=i
    e127 = np.zeros((P, P), np.float32)
    e127[P - 1, :] = 1.0  # lhsT: broadcasts row 127 of rhs to all partitions

    in_maps = []
    for c in range(n_cores):
        sel = np.nonzero(owner == c)[0]
        lab = labels[sel]
        order = np.argsort(lab, kind="stable")
        perm = sel[order]
        lab_sorted = lab[order]
        n = len(perm)

        feats = np.zeros((cap, FEAT_DIM), np.float32)
        feats[:n] = features[perm]

        cg = np.full(cap, shard, np.int32)   # pad -> OOB (skipped)
        hi = np.zeros(cap, np.int32)         # pad -> scratch row 0 (zeros)
        lo = np.zeros(cap, np.int32)
        invh = np.zeros(cap, np.float32)
        if n:
            local = (lab_sorted - c * shard).astype(np.int64)
            newrun = np.r_[True, local[1:] != local[:-1]]
            run_id = np.cumsum(newrun) - 1
            run_start = np.nonzero(newrun)[0]
            run_len = np.diff(np.r_[run_start, n])
            first = run_start[run_id]
            cnt = run_len[run_id]
            cg[:n] = local
            hi[:n] = (first + cnt).astype(np.int32)   # 1 + last index of run
            lo[:n] = first.astype(np.int32)
            invh[:n] = (ALPHA / cnt).astype(np.float32)

        in_maps.append({
            "centers_shard": centers[c * shard:(c + 1) * shard],
            "feats": feats,
            "cg": np.ascontiguousarray(cg.reshape(nch, P).T),
            "hi": np.ascontiguousarray(hi.reshape(nch, P).T),
            "lo": np.ascontiguousarray(lo.reshape(nch, P).T),
            "invh": np.ascontiguousarray(invh.reshape(nch, P).T),
            "tri": tri,
            "e127": e127,
        })
    return in_maps, cap, nch


# ---------------------------------------------------------------------------
# Device program
# ---------------------------------------------------------------------------

def build_program(nch, shard=SHARD, ncopy=8, debug=False, variant="full"):
    """Build the per-core Bass/Tile program (same program for all cores).

    variant: copy | b1a | b1b | noscatter | full (incremental feature sets).
    """
    from concourse import bacc, bass, mybir
    import concourse.tile as tile
    from concourse.tile_rust import add_dep_helper

    f32 = mybir.dt.float32
    i32 = mybir.dt.int32
    cap = nch * P
    D = FEAT_DIM

    nc = bacc.Bacc("TRN2", target_bir_lowering=False, debug=debug,
                   num_devices=N_CORES)

    cs = nc.dram_tensor("centers_shard", [shard, D], f32, kind="ExternalInput").ap()
    feats = nc.dram_tensor("feats", [cap, D], f32, kind="ExternalInput").ap()
    cg = nc.dram_tensor("cg", [P, nch], i32, kind="ExternalInput").ap()
    hi = nc.dram_tensor("hi", [P, nch], i32, kind="ExternalInput").ap()
    lo = nc.dram_tensor("lo", [P, nch], i32, kind="ExternalInput").ap()
    invh = nc.dram_tensor("invh", [P, nch], f32, kind="ExternalInput").ap()
    tri = nc.dram_tensor("tri", [P, P], f32, kind="ExternalInput").ap()
    e127 = nc.dram_tensor("e127", [P, P], f32, kind="ExternalInput").ap()

    out = nc.dram_tensor("out_shard", [shard, D], f32, kind="ExternalOutput").ap()
    loss_part = nc.dram_tensor("loss_part", [P, 1], f32, kind="ExternalOutput").ap()

    scratch = nc.dram_tensor("scratch", [cap + 1, D], f32).ap()  # Internal

    def raw(inst):
        return getattr(inst, "ins", inst)

    with tile.TileContext(nc) as tc, ExitStack() as ctx:
        const = ctx.enter_context(tc.tile_pool(name="const", bufs=1))
        fpool = ctx.enter_context(tc.tile_pool(name="fpool", bufs=4))
        psum = ctx.enter_context(tc.tile_pool(name="psum", bufs=4, space="PSUM"))
        prefp = ctx.enter_context(tc.tile_pool(name="prefp", bufs=4))
        ctp = ctx.enter_context(tc.tile_pool(name="ctp", bufs=3))
        chp = ctx.enter_context(tc.tile_pool(name="chp", bufs=nch))
        wk = ctx.enter_context(tc.tile_pool(name="wk", bufs=3))
        accp = ctx.enter_context(tc.tile_pool(name="accp", bufs=2))

        # --- constants / index tables ---
        tri_t = const.tile([P, P], f32, tag="tri")
        e127_t = const.tile([P, P], f32, tag="e127")
        cg_t = const.tile([P, nch], i32, tag="cg")
        hi_t = const.tile([P, nch], i32, tag="hi")
        lo_t = const.tile([P, nch], i32, tag="lo")
        invh_t = const.tile([P, nch], f32, tag="invh")
        nc.sync.dma_start(out=tri_t[:], in_=tri[:])
        nc.sync.dma_start(out=e127_t[:], in_=e127[:])
        nc.sync.dma_start(out=cg_t[:], in_=cg[:])
        nc.sync.dma_start(out=hi_t[:], in_=hi[:])
        nc.sync.dma_start(out=lo_t[:], in_=lo[:])
        nc.sync.dma_start(out=invh_t[:], in_=invh[:])

        zrow = const.tile([1, D], f32, tag="zrow")
        nc.vector.memset(zrow[:], 0)
        zrow_store = nc.sync.dma_start(out=scratch[0:1, :], in_=zrow[:])

        # --- Phase A: bulk copy shard -> out (DRAM->DRAM) ---
        copy_insts = []
        step = -(-shard // ncopy)
        for k in range(ncopy):
            r0 = k * step
            r1 = min(shard, r0 + step)
            if r0 >= r1:
                break
            inst = nc.sync.dma_start(out=out[r0:r1, :], in_=cs[r0:r1, :])
            copy_insts.append(raw(inst))

        # --- Phase B1: global exclusive-prefix of routed features ---
        acc = accp.tile([P, 1], f32, tag="acc")
        nc.vector.memset(acc[:], 0)


        do_b1 = variant in ("b1a", "b1b", "b1b_nottr", "noscatter", "full")
        do_loss = variant in ("b1b", "b1b_nottr", "noscatter", "full")
        do_bars = variant in ("noscatter", "full")
        do_b2 = variant == "full"

        pref_prev = None
        store_insts = [raw(zrow_store)]
        ch_tiles = []
        for ic in range(nch if do_b1 else 0):
            f_t = fpool.tile([P, D], f32, tag="F")
            nc.sync.dma_start(out=f_t[:], in_=feats[ic * P:(ic + 1) * P, :])

            ps = psum.tile([P, D], f32, tag="ps")
            if ic == 0:
                nc.tensor.matmul(out=ps[:], lhsT=tri_t[:], rhs=f_t[:],
                                 start=True, stop=True)
            else:
                nc.tensor.matmul(out=ps[:], lhsT=tri_t[:], rhs=f_t[:],
                                 start=True, stop=False)
                # += broadcast of pref_prev row 127 (running carry)
                nc.tensor.matmul(out=ps[:], lhsT=e127_t[:], rhs=pref_prev[:],
                                 start=False, stop=True)

            pref = prefp.tile([P, D], f32, tag="pref")
            nc.vector.tensor_copy(out=pref[:], in_=ps[:])
            st = nc.sync.dma_start(out=scratch[1 + ic * P: 1 + (ic + 1) * P, :],
                                   in_=pref[:])
            store_insts.append(raw(st))
            pref_prev = pref

            # gather old centers rows for this chunk (pad -> OOB skipped, stays 0)
            if do_loss:
                ct = ctp.tile([P, D], f32, tag="ct")
                nc.vector.memset(ct[:], 0)
                if do_b2:
                    nc.gpsimd.indirect_dma_start(
                        out=ct[:], out_offset=None,
                        in_=cs[:],
                        in_offset=bass.IndirectOffsetOnAxis(
                            ap=cg_t[:, ic:ic + 1], axis=0),
                        bounds_check=shard - 1, oob_is_err=False,
                    )

                ch = chp.tile([P, D], f32, tag="ch")
                nc.vector.tensor_scalar_mul(out=ch[:], in0=ct[:],
                                            scalar1=1.0 - ALPHA)
                ch_tiles.append(ch)

                # loss: acc += sum_free((F - C)^2)
                d = wk.tile([P, D], f32, tag="d")
                nc.vector.tensor_tensor(out=d[:], in0=f_t[:], in1=ct[:],
                                        op=mybir.AluOpType.subtract)
                # (tensor_tensor_reduce is rejected by this HW path; use
                # mult + reduce + add on DVE instead)
                d2 = wk.tile([P, D], f32, tag="d2")
                acc_new = accp.tile([P, 1], f32, tag="acc")
                nc.vector.tensor_tensor(out=d2[:], in0=d[:], in1=d[:],
                                        op=mybir.AluOpType.mult)
                col = wk.tile([P, 1], f32, tag="col")
                nc.vector.tensor_reduce(out=col[:], in_=d2[:],
                                        axis=mybir.AxisListType.X,
                                        op=mybir.AluOpType.add)
                nc.vector.tensor_tensor(out=acc_new[:], in0=acc[:],
                                        in1=col[:], op=mybir.AluOpType.add)
                acc = acc_new

        if do_bars:
            # barrier: all prefix stores done before boundary gathers
            bar_pref = raw(nc.sync.nop(nofuse=True, hint="prefix_done"))
            for st in store_insts:
                add_dep_helper(bar_pref, st, reason="prefix gathers wait on stores")

            # barrier: bulk copy done before scatters overwrite rows
            bar_copy = raw(nc.sync.nop(nofuse=True, hint="copy_done"))
            for cp in copy_insts:
                add_dep_helper(bar_copy, cp, reason="scatter waits on bulk copy")

        # --- Phase B2: segment means + blended scatter ---
        for ic in range(nch if do_b2 else 0):
            h_t = wk.tile([P, D], f32, tag="H")
            g1 = nc.gpsimd.indirect_dma_start(
                out=h_t[:], out_offset=None, in_=scratch[:],
                in_offset=bass.IndirectOffsetOnAxis(ap=hi_t[:, ic:ic + 1], axis=0))
            l_t = wk.tile([P, D], f32, tag="L")
            g2 = nc.gpsimd.indirect_dma_start(
                out=l_t[:], out_offset=None, in_=scratch[:],
                in_offset=bass.IndirectOffsetOnAxis(ap=lo_t[:, ic:ic + 1], axis=0))
            add_dep_helper(raw(g1), bar_pref, reason="gather hi after stores")
            add_dep_helper(raw(g2), bar_pref, reason="gather lo after stores")

            s_t = wk.tile([P, D], f32, tag="S")
            nc.vector.tensor_tensor(out=s_t[:], in0=h_t[:], in1=l_t[:],
                                    op=mybir.AluOpType.subtract)
            m_t = wk.tile([P, D], f32, tag="MH")
            nc.vector.tensor_scalar_mul(out=m_t[:], in0=s_t[:],
                                        scalar1=invh_t[:, ic:ic + 1])
            b_t = wk.tile([P, D], f32, tag="BL")
            nc.vector.tensor_tensor(out=b_t[:], in0=m_t[:], in1=ch_tiles[ic][:],
                                    op=mybir.AluOpType.add)

            sc = nc.gpsimd.indirect_dma_start(
                out=out[:],
                out_offset=bass.IndirectOffsetOnAxis(ap=cg_t[:, ic:ic + 1], axis=0),
                in_=b_t[:], in_offset=None,
                bounds_check=shard - 1, oob_is_err=False,
            )
            add_dep_helper(raw(sc), bar_copy, reason="scatter after bulk copy")

        nc.sync.dma_start(out=loss_part[:], in_=acc[:])

    nc.compile()
    return nc


def _get_program(nch):
    if nch not in _PROGRAM_CACHE:
        _PROGRAM_CACHE[nch] = build_program(nch)
    return _PROGRAM_CACHE[nch]


# ---------------------------------------------------------------------------
# Entry point
# ---------------------------------------------------------------------------

LAST_RESULTS = None  # populated for profiling harnesses


def kernel(features, centers, labels):
    global LAST_RESULTS
    from concourse.bass_utils import run_bass_kernel_spmd

    in_maps, cap, nch = _route(features, centers, labels)
    nc = _get_program(nch)
    res = run_bass_kernel_spmd(nc, in_maps, list(range(N_CORES)))
    LAST_RESULTS = res

    new_centers = np.concatenate(
        [res.results[c]["out_shard"] for c in range(N_CORES)], axis=0)
    total = sum(float(res.results[c]["loss_part"].sum(dtype=np.float64))
                for c in range(N_CORES))
    loss = np.float32(LAMBDA_C * total / (2.0 * BATCH))
    return loss, new_centers
